# revision 33
# baseline (speedup 1.0000x reference)
"""Trainium2 Bass kernel for nn_Attention_70136815943694.

Attention with the reference's source bug preserved (K uses the V
projection). x:[2,2048,1024], 16 heads x 64 dim. Sharded over 8
NeuronCores as (batch x head-group): core c handles batch c//4 and
heads [4*(c%4) .. 4*(c%4)+3]. Each core's output slice is independent,
so there are no collectives; the host shards inputs and reassembles.

Per-core device pipeline (d-major layouts):
  QT = wqT.T @ xT (+bq)     [256, 2048] bf16   (DVE evac w/ bias)
  KVT = wvT.T @ xT (+bv)    [256, 2048] bf16
  V   = PE-transpose of KVT chunks (bias already included)
  per head-pair p, s1 quarter q (512 wide), s2 chunk j (128):
    scores: two K=64 matmuls row-packed via tile_position (0,0)/(64,0)
    PT = exp(0.125*scores) on ACT, one [128,1024] op for both heads
    atH += [V_h | 1].T @ PT_h   [65, 512] psum, row 64 = softmax denom
  epilogue: gpsimd partition_broadcast denom, DVE fast reciprocal, mul
The attention phase is ACT(exp)-bound; all projection/V work beyond the
minimal prologue (first halves of QT/KVT m=0) is interleaved into the
attention groups as PE filler so the exp stream starts ~as soon as the
input DMAs land and never starves.
"""
import numpy as np
import ml_dtypes

B = 2
S = 2048
D = 1024
NH = 16
HD = 64
N_CORES = 8
HEADS_PER_CORE = 4
DPC = HEADS_PER_CORE * HD  # 256 projection rows per core
P = 128
KC = D // P  # 8 contraction chunks
SC = S // P  # 16 s2 chunks
SQ = 512  # s1 quarter width
NSQ = S // SQ  # 4

_NC_CACHE = {}


def build_nc():
    if "nc" in _NC_CACHE:
        return _NC_CACHE["nc"]
    import concourse.bass as bass
    import concourse.mybir as mybir
    import concourse.tile as tile
    from concourse import bacc
    from concourse.masks import make_identity

    BF16 = mybir.dt.bfloat16
    F32 = mybir.dt.float32
    Act = mybir.ActivationFunctionType
    ts = bass.ts

    nc = bacc.Bacc(None, target_bir_lowering=False, debug=False)
    xT_d = nc.declare_dram_parameter("xT", [D, S], BF16, isOutput=False)
    wqT_d = nc.declare_dram_parameter("wqT", [P, KC, DPC], BF16, isOutput=False)
    wvT_d = nc.declare_dram_parameter("wvT", [P, KC, DPC], BF16, isOutput=False)
    bq_d = nc.declare_dram_parameter("bq", [DPC, 1], F32, isOutput=False)
    bv_d = nc.declare_dram_parameter("bv", [DPC, 1], F32, isOutput=False)
    out_d = nc.declare_dram_parameter("out", [DPC, S], F32, isOutput=True)

    with tile.TileContext(nc) as tc:
        with (
            tc.tile_pool(name="persist", bufs=1) as persist,
            tc.tile_pool(name="pt", bufs=8) as pt_pool,
            tc.tile_pool(name="epi", bufs=4) as epi_pool,
        ):
            # warm the ACT exp table set at t~0 so the one-time table load
            # overlaps the input DMAs
            warm = persist.tile([1, 8], F32, tag="warm")
            nc.vector.memset(warm[:], 0.0)
            nc.scalar.activation(warm[:], warm[:], Act.Exp, scale=1.0)

            ident = persist.tile([P, P], BF16, tag="ident")
            make_identity(nc, ident[:])

            # ---- input loads: xT on the sync HWDGE queue, weights/biases on
            # the gpsimd SWDGE queue so the issue streams run in parallel.
            xt_sb = [
                persist.tile([P, S], BF16, name=f"xt{k}", tag=f"xt{k}")
                for k in range(KC)
            ]
            wq_big = persist.tile([P, KC, DPC], BF16, name="wq", tag="wq")
            wv_big = persist.tile([P, KC, DPC], BF16, name="wv", tag="wv")
            wq_sb = [wq_big[:, k, :] for k in range(KC)]
            wv_sb = [wv_big[:, k, :] for k in range(KC)]
            bq_sb = [
                persist.tile([P, 1], F32, name=f"bq{m}", tag=f"bq{m}")
                for m in range(2)
            ]
            bv_sb = [
                persist.tile([P, 1], F32, name=f"bv{m}", tag=f"bv{m}")
                for m in range(2)
            ]
            # wq0/wv0 first (small, unblock the first LDWEIGHTS), then xT in
            # column-quarters, column-major: the prologue only reads cols
            # 0:512 of every k-chunk, so its 1MB lands in ~3us instead of
            # waiting for the full 4MB.
            nc.sync.dma_start(wq_big[:], wqT_d[:])
            nc.sync.dma_start(wv_big[:], wvT_d[:])
            for k in range(KC):
                nc.sync.dma_start(
                    xt_sb[k][:, 0:512], xT_d[ts(k, P), 0:512]
                )
            for k in range(KC):
                nc.sync.dma_start(
                    xt_sb[k][:, 512:2048], xT_d[ts(k, P), 512:2048]
                )
            for m in range(2):
                nc.gpsimd.dma_start(bq_sb[m][:], bq_d[ts(m, P), :])
                nc.gpsimd.dma_start(bv_sb[m][:], bv_d[ts(m, P), :])

            qT_sb = [
                persist.tile([P, S], BF16, name=f"qT{m}", tag=f"qT{m}")
                for m in range(2)
            ]
            kvT_sb = [
                persist.tile([P, S], BF16, name=f"kvT{m}", tag=f"kvT{m}")
                for m in range(2)
            ]
            # v_sb[p][hl][j]: [128, 65] = V chunk j for head 2p+hl, col 64 = 1
            v_sb = [
                [
                    [
                        persist.tile(
                            [P, HD + 1], BF16,
                            name=f"v{p}_{hl}_{j}", tag=f"v{p}_{hl}_{j}",
                        )
                        for j in range(SC)
                    ]
                    for hl in range(2)
                ]
                for p in range(2)
            ]
            for p in range(2):
                for hl in range(2):
                    for j in range(SC):
                        nc.vector.memset(v_sb[p][hl][j][:, HD : HD + 1], 1.0)

            def proj512(w_sb, dst, bias, m, c0, psum_pool, stepped, warm=0):
                """One 512-col slice [c0:c0+512] of a projection m-chunk.
                warm>0 emits that many throwaway ident matmuls into the psum
                first (overwritten by the real k0 start=True) to lift the PE
                HAM clock gate during the DMA-bound start."""
                ps = psum_pool.tile([P, 512], F32, tag="mi", name="pp")
                nq = c0 // 512
                for _ in range(warm):
                    nc.tensor.matmul(
                        ps[:, 0:P], ident[:], ident[:], start=True, stop=True
                    )
                for k in range(KC):
                    nc.tensor.matmul(
                        ps[:],
                        w_sb[k][:, ts(m, P)],
                        xt_sb[k][:, ts(nq, 512)],
                        start=(k == 0),
                        stop=(k == KC - 1),
                    )
                    if stepped and k % 2 == 1:
                        yield
                nc.vector.tensor_scalar_add(
                    dst[:, ts(nq, 512)], ps[:], bias[:]
                )
                if stepped:
                    yield

            def vtrans_steps(p, psum_pool, j0=0):
                """PE-transpose KVT chunks into natural-layout V tiles."""
                for j in range(j0, SC):
                    pst = psum_pool.tile(
                        [P, P], BF16, tag="mi", name="vt",
                        padded_shape=[P, 1024],
                    )
                    nc.tensor.transpose(
                        pst[:], kvT_sb[p][:, ts(j, P)], ident[:]
                    )
                    for hl in range(2):
                        nc.vector.tensor_copy(
                            v_sb[p][hl][j][:, 0:HD], pst[:, ts(hl, HD)]
                        )
                    if j % 2 == 1:
                        yield

            # ---- prologue: the minimum before exps can flow: qT m0 cols
            # 0:512 ((0,0) scores rhs) and KVT m0 cols 0:1024 (scores lhsT
            # for j<8 plus the first V transposes).
            with tc.tile_pool(name="psum_pro", bufs=4, space="PSUM") as psum_pro:
                wps = psum_pro.tile([P, 512], F32, tag="warm", name="wps")
                for i in range(34):
                    nc.tensor.matmul(
                        wps[:, 0:P], ident[:], ident[:], start=True, stop=True
                    )

                # Q and KV interleaved per k: after the last xT chunk
                # arrives only two matmuls + the evacs remain before the
                # first scores can issue.
                ps_q = psum_pro.tile([P, 512], F32, tag="mi", name="ppq")
                ps_v = psum_pro.tile([P, 512], F32, tag="mi", name="ppv")
                for k in range(KC):
                    nc.tensor.matmul(
                        ps_q[:],
                        wq_sb[k][:, 0:P],
                        xt_sb[k][:, 0:512],
                        start=(k == 0),
                        stop=(k == KC - 1),
                    )
                    nc.tensor.matmul(
                        ps_v[:],
                        wv_sb[k][:, 0:P],
                        xt_sb[k][:, 0:512],
                        start=(k == 0),
                        stop=(k == KC - 1),
                    )
                nc.vector.tensor_scalar_add(qT_sb[0][:, 0:512], ps_q[:], bq_sb[0][:])
                nc.vector.tensor_scalar_add(kvT_sb[0][:, 0:512], ps_v[:], bv_sb[0][:])
                # read wps once so the warm-up matmuls aren't dead code
                nc.vector.tensor_copy(warm[:], wps[0:1, 0:8])

            # ---- attention ---------------------------------------------------
            with (
                tc.tile_pool(name="psum_sc", bufs=2, space="PSUM") as psum_sc,
                tc.tile_pool(name="psum_at", bufs=2, space="PSUM") as psum_at,
                tc.tile_pool(name="psum_mi", bufs=2, space="PSUM") as psum_mi,
            ):
                # Preseed the first two V chunks so (0,0)'s first attnT
                # matmuls have emitted writers (Tile deps follow trace order).
                vt0 = vtrans_steps(0, psum_mi, j0=0)
                next(vt0)  # chunks 0,1

                def adv(g, n):
                    for _ in range(n):
                        try:
                            next(g)
                        except StopIteration:
                            return

                def fill00():
                    """(0,0) filler with explicit RAW-safe ordering: KVT m0
                    col-slices must be emitted before the V transposes (and
                    scores) that read them; V chunk j before attnT-j."""
                    kv512 = proj512(wv_sb, kvT_sb[0], bv_sb[0], 0, 512, psum_mi, True)
                    kv1024 = proj512(wv_sb, kvT_sb[0], bv_sb[0], 0, 1024, psum_mi, True)
                    kv1536 = proj512(wv_sb, kvT_sb[0], bv_sb[0], 0, 1536, psum_mi, True)
                    q512 = proj512(wq_sb, qT_sb[0], bq_sb[0], 0, 512, psum_mi, True)
                    adv(kv512, 2); yield
                    adv(kv512, 2); yield
                    adv(kv512, 1); adv(vt0, 1); yield
                    adv(kv1024, 2); yield
                    adv(kv1024, 2); adv(vt0, 1); yield
                    adv(kv1024, 1); adv(vt0, 1); yield
                    adv(kv1536, 2); adv(vt0, 1); yield
                    adv(kv1536, 2); adv(vt0, 1); yield
                    adv(kv1536, 1); adv(vt0, 1); yield
                    adv(vt0, 1); yield
                    adv(q512, 2); yield
                    adv(q512, 2); yield
                    adv(q512, 1); yield

                # Filler schedule: each piece lands in the latest group that
                # still meets its consumer's deadline, so no group is
                # overloaded and the ACT exp stream stays dense.
                fillers = {
                    (0, 0): [fill00()],
                    (0, 1): [
                        proj512(wq_sb, qT_sb[0], bq_sb[0], 0, 1024, psum_mi, True),
                        proj512(wv_sb, kvT_sb[1], bv_sb[1], 1, 0, psum_mi, True),
                        proj512(wv_sb, kvT_sb[1], bv_sb[1], 1, 512, psum_mi, True),
                    ],
                    (0, 2): [
                        proj512(wq_sb, qT_sb[0], bq_sb[0], 0, 1536, psum_mi, True),
                        proj512(wv_sb, kvT_sb[1], bv_sb[1], 1, 1024, psum_mi, True),
                        proj512(wv_sb, kvT_sb[1], bv_sb[1], 1, 1536, psum_mi, True),
                    ],
                    (0, 3): [
                        proj512(wq_sb, qT_sb[1], bq_sb[1], 1, 0, psum_mi, True),
                        vtrans_steps(1, psum_mi),
                    ],
                    (1, 0): [
                        proj512(wq_sb, qT_sb[1], bq_sb[1], 1, 512, psum_mi, True),
                    ],
                    (1, 1): [
                        proj512(wq_sb, qT_sb[1], bq_sb[1], 1, 1024, psum_mi, True),
                    ],
                    (1, 2): [
                        proj512(wq_sb, qT_sb[1], bq_sb[1], 1, 1536, psum_mi, True),
                    ],
                }

                def emit_epilogue(p, q, at, last=False):
                    for hl in range(2):
                        head = 2 * p + hl
                        asb = epi_pool.tile([HD, SQ], F32, tag="asb", name="asb")
                        nc.vector.tensor_copy(asb[:], at[hl][0:HD, :])
                        # partition_broadcast reads the tensor's partition 0
                        # regardless of AP offset: stage the denom row in a
                        # dedicated p0 tile first.
                        dr = epi_pool.tile([1, SQ], F32, tag="dr", name="dr")
                        nc.vector.tensor_copy(dr[:], at[hl][HD : HD + 1, :])
                        bc = epi_pool.tile([HD, SQ], F32, tag="bc", name="bc")
                        nc.gpsimd.partition_broadcast(bc[:], dr[:])
                        rc = epi_pool.tile([HD, SQ], F32, tag="rc", name="rc")
                        nc.vector.reciprocal_approx_fast(rc[:], bc[:])
                        ot = epi_pool.tile([HD, SQ], F32, tag="ot", name="ot")
                        nc.vector.tensor_mul(ot[:], asb[:], rc[:])
                        nc.sync.dma_start(out_d[ts(head, HD), ts(q, SQ)], ot[:])

                # Software-pipelined: attnT for slot i is emitted during slot
                # i+1, so the next group's scores/exp never sit behind the
                # previous group's last attnT in PE program order.
                slots = [(p, q, j) for p in range(2) for q in range(NSQ)
                         for j in range(SC)]
                gens = []
                at = None
                prev = None
                for p, q, j in slots:
                    if j == 0:
                        gens = fillers.get((p, q), []) + gens
                        at = [
                            psum_at.tile([HD + 1, SQ], F32, tag="at", name="at")
                            for _ in range(2)
                        ]
                    sc = psum_sc.tile([P, 1024], F32, tag="sc", name="sc")
                    for hl in range(2):
                        nc.tensor.matmul(
                            sc[:, ts(hl, SQ)],
                            kvT_sb[p][hl * HD : (hl + 1) * HD, ts(j, P)],
                            qT_sb[p][hl * HD : (hl + 1) * HD, ts(q, SQ)],
                            start=True,
                            stop=True,
                            tile_position=(hl * HD, 0),
                        )
                    pt = pt_pool.tile([P, 1024], BF16, tag="pt", name="pt")
                    nc.scalar.activation(pt[:], sc[:], Act.Exp, scale=0.125)
                    # filler work (remaining projections, V transposes); one
                    # step per slot keeps PE bursts smaller than the exp time
                    if gens:
                        g = gens.pop(0)
                        try:
                            next(g)
                            gens.append(g)
                        except StopIteration:
                            pass
                    if prev is not None:
                        pp, pq, pj, pat, ppt = prev
                        for hl in range(2):
                            nc.tensor.matmul(
                                pat[hl][:],
                                v_sb[pp][hl][pj][:],
                                ppt[:, ts(hl, SQ)],
                                start=(pj == 0),
                                stop=(pj == SC - 1),
                            )
                        if pj == SC - 1:
                            emit_epilogue(pp, pq, pat)
                    prev = (p, q, j, at, pt)
                # flush the last slot
                p, q, j, at, pt = prev
                for hl in range(2):
                    nc.tensor.matmul(
                        at[hl][:],
                        v_sb[p][hl][j][:],
                        pt[:, ts(hl, SQ)],
                        start=False,
                        stop=True,
                    )
                emit_epilogue(p, q, at, last=True)

    nc.compile()
    _NC_CACHE["nc"] = nc
    return nc


def shard_inputs(x, Wq, bq, Wv, bv):
    bf16 = ml_dtypes.bfloat16
    x = np.asarray(x, dtype=np.float32)
    Wq = np.asarray(Wq, dtype=np.float32)
    bq = np.asarray(bq, dtype=np.float32)
    Wv = np.asarray(Wv, dtype=np.float32)
    bv = np.asarray(bv, dtype=np.float32)
    in_maps = []
    xT = [np.ascontiguousarray(x[b].T).astype(bf16) for b in range(B)]
    for c in range(N_CORES):
        b, g = divmod(c, N_CORES // B)
        heads = [HEADS_PER_CORE * g + hl for hl in range(HEADS_PER_CORE)]
        perm = np.array([i * NH + h for h in heads for i in range(HD)])
        in_maps.append(
            {
                "xT": xT[b],
                "wqT": np.ascontiguousarray(
                    Wq[perm, :].T.reshape(KC, P, DPC).transpose(1, 0, 2)
                ).astype(bf16),
                "wvT": np.ascontiguousarray(
                    Wv[perm, :].T.reshape(KC, P, DPC).transpose(1, 0, 2)
                ).astype(bf16),
                "bq": np.ascontiguousarray(bq[perm].reshape(DPC, 1)),
                "bv": np.ascontiguousarray(bv[perm].reshape(DPC, 1)),
            }
        )
    return in_maps


def assemble(results):
    out = np.empty((B, S, D), dtype=np.float32)
    for c in range(N_CORES):
        b, g = divmod(c, N_CORES // B)
        out[b][:, g * DPC : (g + 1) * DPC] = results[c]["out"].T
    return out


def kernel(x, Wq, bq, Wv, bv):
    from concourse.bass_utils import run_bass_kernel_spmd

    nc = build_nc()
    in_maps = shard_inputs(x, Wq, bq, Wv, bv)
    res = run_bass_kernel_spmd(nc, in_maps, core_ids=list(range(N_CORES)))
    return assemble(res.results)


if __name__ == "__main__":
    rng = np.random.default_rng(0)
    inputs = {
        "x": rng.standard_normal((B, S, D), dtype=np.float32),
        "Wq": (rng.standard_normal((D, D), dtype=np.float32) / 32.0),
        "bq": rng.standard_normal(D, dtype=np.float32) * 0.02,
        "Wv": (rng.standard_normal((D, D), dtype=np.float32) / 32.0),
        "bv": rng.standard_normal(D, dtype=np.float32) * 0.02,
    }
    out = kernel(**inputs)
    print("kernel ran, out shape:", out.shape)


# revision 34
# speedup vs baseline: 1.0161x; 1.0161x over previous
"""Trainium2 Bass kernel for nn_Attention_70136815943694.

Attention with the reference's source bug preserved (K uses the V
projection). x:[2,2048,1024], 16 heads x 64 dim. Sharded over 8
NeuronCores as (batch x head-group): core c handles batch c//4 and
heads [4*(c%4) .. 4*(c%4)+3]. Each core's output slice is independent,
so there are no collectives; the host shards inputs and reassembles.

Per-core device pipeline (d-major layouts):
  QT = wqT.T @ xT (+bq)     [256, 2048] bf16   (DVE evac w/ bias)
  KVT = wvT.T @ xT (+bv)    [256, 2048] bf16
  V   = PE-transpose of KVT chunks (bias already included)
  per head-pair p, s1 quarter q (512 wide), s2 chunk j (128):
    scores: two K=64 matmuls row-packed via tile_position (0,0)/(64,0)
    PT = exp(0.125*scores) on ACT, one [128,1024] op for both heads
    atH += [V_h | 1].T @ PT_h   [65, 512] psum, row 64 = softmax denom
  epilogue: gpsimd partition_broadcast denom, DVE fast reciprocal, mul
The attention phase is ACT(exp)-bound; all projection/V work beyond the
minimal prologue (first halves of QT/KVT m=0) is interleaved into the
attention groups as PE filler so the exp stream starts ~as soon as the
input DMAs land and never starves.
"""
import numpy as np
import ml_dtypes

B = 2
S = 2048
D = 1024
NH = 16
HD = 64
N_CORES = 8
HEADS_PER_CORE = 4
DPC = HEADS_PER_CORE * HD  # 256 projection rows per core
P = 128
KC = D // P  # 8 contraction chunks
SC = S // P  # 16 s2 chunks
SQ = 512  # s1 quarter width
NSQ = S // SQ  # 4

_NC_CACHE = {}


def build_nc():
    if "nc" in _NC_CACHE:
        return _NC_CACHE["nc"]
    import concourse.bass as bass
    import concourse.mybir as mybir
    import concourse.tile as tile
    from concourse import bacc
    from concourse.masks import make_identity

    BF16 = mybir.dt.bfloat16
    F32 = mybir.dt.float32
    Act = mybir.ActivationFunctionType
    ts = bass.ts

    nc = bacc.Bacc(None, target_bir_lowering=False, debug=False)
    xT_d = nc.declare_dram_parameter("xT", [D, S], BF16, isOutput=False)
    wqT_d = nc.declare_dram_parameter("wqT", [P, KC, DPC], BF16, isOutput=False)
    wvT_d = nc.declare_dram_parameter("wvT", [P, KC, DPC], BF16, isOutput=False)
    bq_d = nc.declare_dram_parameter("bq", [DPC, 1], F32, isOutput=False)
    bv_d = nc.declare_dram_parameter("bv", [DPC, 1], F32, isOutput=False)
    out_d = nc.declare_dram_parameter("out", [DPC, S], F32, isOutput=True)

    with tile.TileContext(nc) as tc:
        with (
            tc.tile_pool(name="persist", bufs=1) as persist,
            tc.tile_pool(name="pt", bufs=8) as pt_pool,
            tc.tile_pool(name="epi", bufs=2) as epi_pool,
        ):
            # warm the ACT exp table set at t~0 so the one-time table load
            # overlaps the input DMAs
            warm = persist.tile([1, 8], F32, tag="warm")
            nc.vector.memset(warm[:], 0.0)
            nc.scalar.activation(warm[:], warm[:], Act.Exp, scale=1.0)

            ident = persist.tile([P, P], BF16, tag="ident")
            make_identity(nc, ident[:])

            # ---- input loads: xT on the sync HWDGE queue, weights/biases on
            # the gpsimd SWDGE queue so the issue streams run in parallel.
            xt_sb = [
                persist.tile([P, S], BF16, name=f"xt{k}", tag=f"xt{k}")
                for k in range(KC)
            ]
            wq_big = persist.tile([P, KC, DPC], BF16, name="wq", tag="wq")
            wv_big = persist.tile([P, KC, DPC], BF16, name="wv", tag="wv")
            wq_sb = [wq_big[:, k, :] for k in range(KC)]
            wv_sb = [wv_big[:, k, :] for k in range(KC)]
            bq_sb = [
                persist.tile([P, 1], F32, name=f"bq{m}", tag=f"bq{m}")
                for m in range(2)
            ]
            bv_sb = [
                persist.tile([P, 1], F32, name=f"bv{m}", tag=f"bv{m}")
                for m in range(2)
            ]
            # wq0/wv0 first (small, unblock the first LDWEIGHTS), then xT in
            # column-quarters, column-major: the prologue only reads cols
            # 0:512 of every k-chunk, so its 1MB lands in ~3us instead of
            # waiting for the full 4MB.
            nc.sync.dma_start(wq_big[:], wqT_d[:])
            nc.sync.dma_start(wv_big[:], wvT_d[:])
            for cq in range(2):
                for k in range(KC):
                    nc.sync.dma_start(
                        xt_sb[k][:, ts(cq, 512)], xT_d[ts(k, P), ts(cq, 512)]
                    )
            for k in range(KC):
                nc.sync.dma_start(
                    xt_sb[k][:, 1024:2048], xT_d[ts(k, P), 1024:2048]
                )
            for m in range(2):
                nc.gpsimd.dma_start(bq_sb[m][:], bq_d[ts(m, P), :])
                nc.gpsimd.dma_start(bv_sb[m][:], bv_d[ts(m, P), :])

            qT_sb = [
                persist.tile([P, S], BF16, name=f"qT{m}", tag=f"qT{m}")
                for m in range(2)
            ]
            kvT_sb = [
                persist.tile([P, S], BF16, name=f"kvT{m}", tag=f"kvT{m}")
                for m in range(2)
            ]
            # v_sb[p][hl][j]: [128, 65] = V chunk j for head 2p+hl, col 64 = 1
            v_sb = [
                [
                    [
                        persist.tile(
                            [P, HD + 1], BF16,
                            name=f"v{p}_{hl}_{j}", tag=f"v{p}_{hl}_{j}",
                        )
                        for j in range(SC)
                    ]
                    for hl in range(2)
                ]
                for p in range(2)
            ]
            for p in range(2):
                for hl in range(2):
                    for j in range(SC):
                        nc.vector.memset(v_sb[p][hl][j][:, HD : HD + 1], 1.0)

            def proj512(w_sb, dst, bias, m, c0, psum_pool, stepped, warm=0):
                """One 512-col slice [c0:c0+512] of a projection m-chunk.
                warm>0 emits that many throwaway ident matmuls into the psum
                first (overwritten by the real k0 start=True) to lift the PE
                HAM clock gate during the DMA-bound start."""
                ps = psum_pool.tile([P, 512], F32, tag="mi", name="pp")
                nq = c0 // 512
                for _ in range(warm):
                    nc.tensor.matmul(
                        ps[:, 0:P], ident[:], ident[:], start=True, stop=True
                    )
                for k in range(KC):
                    nc.tensor.matmul(
                        ps[:],
                        w_sb[k][:, ts(m, P)],
                        xt_sb[k][:, ts(nq, 512)],
                        start=(k == 0),
                        stop=(k == KC - 1),
                    )
                    if stepped and k % 2 == 1:
                        yield
                nc.vector.tensor_scalar_add(
                    dst[:, ts(nq, 512)], ps[:], bias[:]
                )
                if stepped:
                    yield

            def vtrans_steps(p, psum_pool, j0=0):
                """PE-transpose KVT chunks into natural-layout V tiles."""
                for j in range(j0, SC):
                    pst = psum_pool.tile(
                        [P, P], BF16, tag="mi", name="vt",
                        padded_shape=[P, 1024],
                    )
                    nc.tensor.transpose(
                        pst[:], kvT_sb[p][:, ts(j, P)], ident[:]
                    )
                    for hl in range(2):
                        nc.vector.tensor_copy(
                            v_sb[p][hl][j][:, 0:HD], pst[:, ts(hl, HD)]
                        )
                    if j % 2 == 1:
                        yield

            # ---- prologue: the minimum before exps can flow: qT m0 cols
            # 0:512 ((0,0) scores rhs) and KVT m0 cols 0:1024 (scores lhsT
            # for j<8 plus the first V transposes).
            with tc.tile_pool(name="psum_pro", bufs=4, space="PSUM") as psum_pro:
                wps = psum_pro.tile([P, 512], F32, tag="warm", name="wps")
                for i in range(34):
                    nc.tensor.matmul(
                        wps[:, 0:P], ident[:], ident[:], start=True, stop=True
                    )

                # Q and KV interleaved per k: after the last xT chunk
                # arrives only two matmuls + the evacs remain before the
                # first scores can issue.
                ps_q = psum_pro.tile([P, 512], F32, tag="mi", name="ppq")
                ps_v = psum_pro.tile([P, 512], F32, tag="mi", name="ppv")
                for k in range(KC):
                    nc.tensor.matmul(
                        ps_q[:],
                        wq_sb[k][:, 0:P],
                        xt_sb[k][:, 0:512],
                        start=(k == 0),
                        stop=(k == KC - 1),
                    )
                    nc.tensor.matmul(
                        ps_v[:],
                        wv_sb[k][:, 0:P],
                        xt_sb[k][:, 0:512],
                        start=(k == 0),
                        stop=(k == KC - 1),
                    )
                nc.vector.tensor_scalar_add(qT_sb[0][:, 0:512], ps_q[:], bq_sb[0][:])
                nc.vector.tensor_scalar_add(kvT_sb[0][:, 0:512], ps_v[:], bv_sb[0][:])
                # read wps once so the warm-up matmuls aren't dead code
                nc.vector.tensor_copy(warm[:], wps[0:1, 0:8])

            # ---- attention ---------------------------------------------------
            with (
                tc.tile_pool(name="psum_sc", bufs=2, space="PSUM") as psum_sc,
                tc.tile_pool(name="psum_at", bufs=2, space="PSUM") as psum_at,
                tc.tile_pool(name="psum_mi", bufs=2, space="PSUM") as psum_mi,
            ):
                # Preseed the first two V chunks so (0,0)'s first attnT
                # matmuls have emitted writers (Tile deps follow trace order).
                vt0 = vtrans_steps(0, psum_mi, j0=0)
                next(vt0)  # chunks 0,1

                def adv(g, n):
                    for _ in range(n):
                        try:
                            next(g)
                        except StopIteration:
                            return

                def fill00():
                    """(0,0) filler with explicit RAW-safe ordering: KVT m0
                    col-slices must be emitted before the V transposes (and
                    scores) that read them; V chunk j before attnT-j."""
                    kv512 = proj512(wv_sb, kvT_sb[0], bv_sb[0], 0, 512, psum_mi, True)
                    kv1024 = proj512(wv_sb, kvT_sb[0], bv_sb[0], 0, 1024, psum_mi, True)
                    kv1536 = proj512(wv_sb, kvT_sb[0], bv_sb[0], 0, 1536, psum_mi, True)
                    q512 = proj512(wq_sb, qT_sb[0], bq_sb[0], 0, 512, psum_mi, True)
                    adv(kv512, 2); yield
                    adv(kv512, 2); yield
                    adv(kv512, 1); adv(vt0, 1); yield
                    adv(kv1024, 2); yield
                    adv(kv1024, 2); adv(vt0, 1); yield
                    adv(kv1024, 1); adv(vt0, 1); yield
                    adv(kv1536, 2); adv(vt0, 1); yield
                    adv(kv1536, 2); adv(vt0, 1); yield
                    adv(kv1536, 1); adv(vt0, 1); yield
                    adv(vt0, 1); yield
                    adv(q512, 2); yield
                    adv(q512, 2); yield
                    adv(q512, 1); yield

                # Filler schedule: each piece lands in the latest group that
                # still meets its consumer's deadline, so no group is
                # overloaded and the ACT exp stream stays dense.
                fillers = {
                    (0, 0): [fill00()],
                    (0, 1): [
                        proj512(wq_sb, qT_sb[0], bq_sb[0], 0, 1024, psum_mi, True),
                        proj512(wv_sb, kvT_sb[1], bv_sb[1], 1, 0, psum_mi, True),
                        proj512(wv_sb, kvT_sb[1], bv_sb[1], 1, 512, psum_mi, True),
                    ],
                    (0, 2): [
                        proj512(wq_sb, qT_sb[0], bq_sb[0], 0, 1536, psum_mi, True),
                        proj512(wv_sb, kvT_sb[1], bv_sb[1], 1, 1024, psum_mi, True),
                        proj512(wv_sb, kvT_sb[1], bv_sb[1], 1, 1536, psum_mi, True),
                    ],
                    (0, 3): [
                        proj512(wq_sb, qT_sb[1], bq_sb[1], 1, 0, psum_mi, True),
                        vtrans_steps(1, psum_mi),
                    ],
                    (1, 0): [
                        proj512(wq_sb, qT_sb[1], bq_sb[1], 1, 512, psum_mi, True),
                    ],
                    (1, 1): [
                        proj512(wq_sb, qT_sb[1], bq_sb[1], 1, 1024, psum_mi, True),
                    ],
                    (1, 2): [
                        proj512(wq_sb, qT_sb[1], bq_sb[1], 1, 1536, psum_mi, True),
                    ],
                }

                def emit_epilogue(p, q, at, last=False):
                    for hl in range(2):
                        head = 2 * p + hl
                        asb = epi_pool.tile([HD, SQ], F32, tag="asb", name="asb")
                        nc.vector.tensor_copy(asb[:], at[hl][0:HD, :])
                        # partition_broadcast reads the tensor's partition 0
                        # regardless of AP offset: stage the denom row in a
                        # dedicated p0 tile first.
                        dr = epi_pool.tile([1, SQ], F32, tag="dr", name="dr")
                        nc.vector.tensor_copy(dr[:], at[hl][HD : HD + 1, :])
                        bc = epi_pool.tile([HD, SQ], F32, tag="bc", name="bc")
                        nc.gpsimd.partition_broadcast(bc[:], dr[:])
                        rc = epi_pool.tile([HD, SQ], F32, tag="rc", name="rc")
                        nc.vector.reciprocal_approx_fast(rc[:], bc[:])
                        ot = epi_pool.tile([HD, SQ], F32, tag="ot", name="ot")
                        nc.vector.tensor_mul(ot[:], asb[:], rc[:])
                        nc.sync.dma_start(out_d[ts(head, HD), ts(q, SQ)], ot[:])

                # Software-pipelined: attnT for slot i is emitted during slot
                # i+1, so the next group's scores/exp never sit behind the
                # previous group's last attnT in PE program order.
                slots = [(p, q, j) for p in range(2) for q in range(NSQ)
                         for j in range(SC)]
                gens = []
                at = None
                prev = None
                for p, q, j in slots:
                    if j == 0:
                        gens = fillers.get((p, q), []) + gens
                        at = [
                            psum_at.tile([HD + 1, SQ], F32, tag="at", name="at")
                            for _ in range(2)
                        ]
                    sc = psum_sc.tile([P, 1024], F32, tag="sc", name="sc")
                    for hl in range(2):
                        nc.tensor.matmul(
                            sc[:, ts(hl, SQ)],
                            kvT_sb[p][hl * HD : (hl + 1) * HD, ts(j, P)],
                            qT_sb[p][hl * HD : (hl + 1) * HD, ts(q, SQ)],
                            start=True,
                            stop=True,
                            tile_position=(hl * HD, 0),
                        )
                    pt = pt_pool.tile([P, 1024], BF16, tag="pt", name="pt")
                    nc.scalar.activation(pt[:], sc[:], Act.Exp, scale=0.125)
                    # filler work (remaining projections, V transposes); one
                    # step per slot keeps PE bursts smaller than the exp time
                    if gens:
                        g = gens.pop(0)
                        try:
                            next(g)
                            gens.append(g)
                        except StopIteration:
                            pass
                    if prev is not None:
                        pp, pq, pj, pat, ppt = prev
                        for hl in range(2):
                            nc.tensor.matmul(
                                pat[hl][:],
                                v_sb[pp][hl][pj][:],
                                ppt[:, ts(hl, SQ)],
                                start=(pj == 0),
                                stop=(pj == SC - 1),
                            )
                        if pj == SC - 1:
                            emit_epilogue(pp, pq, pat)
                    prev = (p, q, j, at, pt)
                # flush the last slot
                p, q, j, at, pt = prev
                for hl in range(2):
                    nc.tensor.matmul(
                        at[hl][:],
                        v_sb[p][hl][j][:],
                        pt[:, ts(hl, SQ)],
                        start=False,
                        stop=True,
                    )
                emit_epilogue(p, q, at, last=True)

    nc.compile()
    _NC_CACHE["nc"] = nc
    return nc


def shard_inputs(x, Wq, bq, Wv, bv):
    bf16 = ml_dtypes.bfloat16
    x = np.asarray(x, dtype=np.float32)
    Wq = np.asarray(Wq, dtype=np.float32)
    bq = np.asarray(bq, dtype=np.float32)
    Wv = np.asarray(Wv, dtype=np.float32)
    bv = np.asarray(bv, dtype=np.float32)
    in_maps = []
    xT = [np.ascontiguousarray(x[b].T).astype(bf16) for b in range(B)]
    for c in range(N_CORES):
        b, g = divmod(c, N_CORES // B)
        heads = [HEADS_PER_CORE * g + hl for hl in range(HEADS_PER_CORE)]
        perm = np.array([i * NH + h for h in heads for i in range(HD)])
        in_maps.append(
            {
                "xT": xT[b],
                "wqT": np.ascontiguousarray(
                    Wq[perm, :].T.reshape(KC, P, DPC).transpose(1, 0, 2)
                ).astype(bf16),
                "wvT": np.ascontiguousarray(
                    Wv[perm, :].T.reshape(KC, P, DPC).transpose(1, 0, 2)
                ).astype(bf16),
                "bq": np.ascontiguousarray(bq[perm].reshape(DPC, 1)),
                "bv": np.ascontiguousarray(bv[perm].reshape(DPC, 1)),
            }
        )
    return in_maps


def assemble(results):
    out = np.empty((B, S, D), dtype=np.float32)
    for c in range(N_CORES):
        b, g = divmod(c, N_CORES // B)
        out[b][:, g * DPC : (g + 1) * DPC] = results[c]["out"].T
    return out


def kernel(x, Wq, bq, Wv, bv):
    from concourse.bass_utils import run_bass_kernel_spmd

    nc = build_nc()
    in_maps = shard_inputs(x, Wq, bq, Wv, bv)
    res = run_bass_kernel_spmd(nc, in_maps, core_ids=list(range(N_CORES)))
    return assemble(res.results)


if __name__ == "__main__":
    rng = np.random.default_rng(0)
    inputs = {
        "x": rng.standard_normal((B, S, D), dtype=np.float32),
        "Wq": (rng.standard_normal((D, D), dtype=np.float32) / 32.0),
        "bq": rng.standard_normal(D, dtype=np.float32) * 0.02,
        "Wv": (rng.standard_normal((D, D), dtype=np.float32) / 32.0),
        "bv": rng.standard_normal(D, dtype=np.float32) * 0.02,
    }
    out = kernel(**inputs)
    print("kernel ran, out shape:", out.shape)
